# revision 17
# baseline (speedup 1.0000x reference)
"""Trainium2 Bass kernel for nn_AttentionTSSA.

Contract: kernel(**inputs) takes FULL inputs (queries [8,1024,8,64] f32,
temp [8,1] f32) and returns the FULL outputs (out [8,1024,8,64],
attn_reshaped [8,1024,8,1024]) matching reference.reference().

Sharding: batch B=8 across the 8 NeuronCores (pure data parallel, no
collectives). Each core computes one batch element.

Per-core algorithm (n = token index 0..1023, h = head, d = feature):
  w[n, h, d]      = queries[b][n, h, d]               (natural layout)
  wsq             = w*w
  colsumsq[h,d]   = sum_n wsq          (PE ones-matvec -> PSUM [1,512])
  invcol_t[h,d]   = temp[h] / max(colsumsq, 1e-24)
  energy[n,h]     = sum_d wsq[n,h,d] * invcol_t[h,d]  (DVE mul + windowed reduce)
  Pi[n,h]         = softmax_h(energy)                 (DVE/ACT, free-dim softmax)
  S[h]            = sum_n Pi            (PE matvec)
  dots[h,d]       = (sum_n Pi[n,h] wsq[n,h,d]) / (S[h]+1e-8)   (PE matmul, diag blocks)
  attn1[h,d]      = 1/(1+dots)
  out[n,h,d]      = -(w*Pi)*attn1      (DVE scalar_tensor_tensor, fused)
  u[n,h,d]        = w / max(sqrt(sum_d wsq), 1e-12)   (bf16)
  G[h]            = (u[h] @ u[h]^T + 1)*0.5           (PE bf16 matmuls; affine
                                                       folded into PSUM->SBUF copy)
  attn_b[n,h,m]   = G[h][n,m]
"""
import sys
import os
from contextlib import ExitStack

sys.path.insert(0, "/opt/trn_rl_repo")

import numpy as np

import concourse.bass as bass
import concourse.bacc as bacc
import concourse.tile as tile
from concourse import mybir
from concourse.masks import make_identity

F32 = mybir.dt.float32
BF16 = mybir.dt.bfloat16

B, L, H, E = 8, 1024, 8, 64
P = 128                 # SBUF partitions per token chunk
NC_CHUNKS = L // P      # 8 chunks of 128 tokens
HD = H * E              # 512
N_CORES = 8

Alu = mybir.AluOpType
Act = mybir.ActivationFunctionType


def build_kernel():
    nc = bacc.Bacc()
    q_d = nc.declare_dram_parameter("queries", [L, H, E], F32, isOutput=False)
    t_d = nc.declare_dram_parameter("temp", [H, 1], F32, isOutput=False)
    out_d = nc.declare_dram_parameter("out", [L, H, E], F32, isOutput=True)
    attn_d = nc.declare_dram_parameter("attn", [L, H, L], F32, isOutput=True)

    with tile.TileContext(nc) as tc, ExitStack() as ctx:
        _body(ctx, tc, q_d, t_d, out_d, attn_d)
    nc.finalize()
    return nc


def _body(ctx, tc, q_d, t_d, out_d, attn_d):
    nc = tc.nc

    const = ctx.enter_context(tc.tile_pool(name="const", bufs=1))
    persist = ctx.enter_context(tc.tile_pool(name="persist", bufs=1))
    work = ctx.enter_context(tc.tile_pool(name="work", bufs=4))
    small = ctx.enter_context(tc.tile_pool(name="small", bufs=6))
    gsb = ctx.enter_context(tc.tile_pool(name="gsb", bufs=6))
    outp = ctx.enter_context(tc.tile_pool(name="outp", bufs=3))
    accps = ctx.enter_context(tc.tile_pool(name="accps", bufs=1, space="PSUM"))

    # ---------------- constants ----------------
    ident = const.tile([P, P], BF16)
    make_identity(nc, ident[:])
    ones128b = const.tile([P, 1], BF16)
    nc.vector.memset(ones128b[:], 1.0)
    ones128_8b = const.tile([P, H], BF16)
    nc.vector.memset(ones128_8b[:], 1.0)
    halfcol = const.tile([P, 1], F32)
    nc.vector.memset(halfcol[:], 0.5)
    # block-diag 0/1 mask on [8, 512] (head block h lives on partition h)
    maskbd = const.tile([H, HD], F32)
    nc.vector.memset(maskbd[:], 1.0)
    nc.gpsimd.affine_select(
        out=maskbd[:], in_=maskbd[:], compare_op=Alu.is_equal, fill=0.0,
        base=0, pattern=[[1, H], [0, E]], channel_multiplier=-1)
    # selector banks: sel[:, h*128:(h+1)*128] = 1 on partition h, else 0.
    # matmul(sel_h, rhs[8,64]) broadcasts rhs row h to all 128 partitions.
    sel = const.tile([H, H * P], F32)
    nc.vector.memset(sel[:], 1.0)
    nc.gpsimd.affine_select(
        out=sel[:], in_=sel[:], compare_op=Alu.is_equal, fill=0.0,
        base=0, pattern=[[1, H], [0, P]], channel_multiplier=-1)
    negsel = const.tile([H, H * P], F32)
    nc.vector.memset(negsel[:], -1.0)
    nc.gpsimd.affine_select(
        out=negsel[:], in_=negsel[:], compare_op=Alu.is_equal, fill=0.0,
        base=0, pattern=[[1, H], [0, P]], channel_multiplier=-1)
    temp_sb = const.tile([H, 1], F32)
    nc.sync.dma_start(temp_sb[:], t_d[:, :])

    # persistent tensors (per-chunk tiles to keep dep tracking fine-grained)
    w_c = [persist.tile([P, HD], F32, tag=f"w{c}", name=f"w{c}") for c in range(NC_CHUNKS)]
    wsq_c = [persist.tile([P, HD], BF16, tag=f"wsq{c}", name=f"wsq{c}") for c in range(NC_CHUNKS)]
    pi_c = [persist.tile([P, H], F32, tag=f"pi{c}", name=f"pi{c}") for c in range(NC_CHUNKS)]
    pib_c = [persist.tile([P, H], BF16, tag=f"pib{c}", name=f"pib{c}") for c in range(NC_CHUNKS)]
    uT_all = persist.tile([E, H * L], BF16, tag="uT")   # [64, 8192]
    uTv = uT_all[:].rearrange("p (h m) -> p h m", m=L)
    invcolb = persist.tile([P, HD], F32, tag="invcolb")
    negattn1b = persist.tile([P, HD], F32, tag="negattn1b")

    # ---------------- pass 1: load, norms, transposed bf16 u ----------------
    # colsumsq accumulated with M=8 so the result lands on 8 partitions
    # (identical rows); the per-head diag blocks are then extracted to [8,64].
    colacc8 = accps.tile([H, HD], F32, tag="acc")
    with tc.tile_pool(name="trps", bufs=3, space="PSUM") as trps:
        for c in range(NC_CHUNKS):
            nc.sync.dma_start(
                w_c[c][:].rearrange("p (h d) -> p h d", d=E),
                q_d[c * P:(c + 1) * P, :, :],
            )
            nc.scalar.square(wsq_c[c][:], w_c[c][:])
            nc.tensor.matmul(colacc8[:], ones128_8b[:], wsq_c[c][:],
                             start=(c == 0), stop=(c == NC_CHUNKS - 1))
            # row sumsq over d per head -> [128, 8]
            rs = small.tile([P, H], F32, tag="rs")
            nc.vector.tensor_reduce(
                rs[:], wsq_c[c][:].rearrange("p (h d) -> p h d", d=E),
                axis=mybir.AxisListType.X, op=Alu.add)
            nc.scalar.sqrt(rs[:], rs[:])
            nc.vector.tensor_scalar_max(rs[:], rs[:], 1e-12)
            inr = small.tile([P, H], F32, tag="inr")
            nc.vector.reciprocal(inr[:], rs[:])
            # u' = w * invrow (bf16), per head window; split DVE/GpSimd
            ub = work.tile([P, HD], BF16, tag="ub")
            for h in range(H):
                nc.vector.tensor_scalar_mul(
                    ub[:, h * E:(h + 1) * E], w_c[c][:, h * E:(h + 1) * E],
                    inr[:, h:h + 1])
            # transpose each [128, 64] head slice -> PSUM [64, 128]
            trt = trps.tile([E, H * P], BF16, tag="tr")   # [64, 1024]
            for h in range(H):
                nc.tensor.transpose(trt[:, h * P:(h + 1) * P],
                                    ub[:, h * E:(h + 1) * E], ident[:])
            # one strided copy: PSUM [64, 8, 128] -> uT_all[:, h, c*128:+128]
            nc.scalar.activation(
                uTv[:, :, c * P:(c + 1) * P],
                trt[:].rearrange("p (h m) -> p h m", m=P),
                Act.Copy)

    # ---------------- invcol on [8,64] + broadcast ----------------
    gps = ctx.enter_context(tc.tile_pool(name="gps", bufs=5, space="PSUM"))

    # extract diag blocks: mask then strided reduce over the head-block axis
    s8 = const.tile([H, HD], F32)
    nc.scalar.activation(s8[:], colacc8[:], Act.Copy)
    nc.vector.tensor_mul(s8[:], s8[:], maskbd[:])
    c8 = const.tile([H, E], F32)
    nc.vector.tensor_reduce(
        c8[:], s8[:].rearrange("p (hb d) -> p d hb", d=E),
        axis=mybir.AxisListType.X, op=Alu.add)
    nc.vector.tensor_scalar_max(c8[:], c8[:], 1e-24)
    nc.vector.reciprocal(c8[:], c8[:])
    nc.vector.tensor_scalar_mul(c8[:], c8[:], temp_sb[:, 0:1])   # fold temp
    # broadcast row h to all partitions of column block h (8 selector matmuls)
    bc_ps = accps.tile([P, HD], F32, tag="bcast")
    for h in range(H):
        nc.tensor.matmul(bc_ps[:, h * E:(h + 1) * E],
                         sel[:, h * P:(h + 1) * P], c8[:],
                         start=True, stop=True)
    nc.scalar.activation(invcolb[:], bc_ps[:], Act.Copy)

    # ---------------- interleaved work emitted inside the G loop ----------
    sacc = accps.tile([H, HD], F32, tag="acc")
    dacc = accps.tile([H, HD], F32, tag="dacc")

    def emit_pass2_chunk(c):
        # energy + softmax over heads for one token chunk
        et = work.tile([P, HD], F32, tag="et")
        nc.gpsimd.tensor_mul(et[:], wsq_c[c][:], invcolb[:])
        en = small.tile([P, H], F32, tag="en")
        nc.vector.tensor_reduce(
            en[:], et[:].rearrange("p (h d) -> p h d", d=E),
            axis=mybir.AxisListType.X, op=Alu.add)
        nmx = small.tile([P, 1], F32, tag="nmx")
        nc.vector.tensor_reduce(nmx[:], en[:], axis=mybir.AxisListType.X,
                                op=Alu.max, negate=True)
        rsum = small.tile([P, 1], F32, tag="rsum")
        nc.scalar.activation(pi_c[c][:], en[:], Act.Exp,
                             bias=nmx[:, 0:1], scale=1.0, accum_out=rsum[:])
        rinv = small.tile([P, 1], F32, tag="rinv")
        nc.vector.reciprocal(rinv[:], rsum[:])
        nc.vector.tensor_scalar_mul(pi_c[c][:], pi_c[c][:], rinv[:, 0:1])
        nc.vector.tensor_copy(pib_c[c][:], pi_c[c][:])

    def emit_sdacc(c):
        nc.tensor.matmul(sacc[:, 0:1], pib_c[c][:], ones128b[:, 0:1],
                         start=(c == 0), stop=(c == NC_CHUNKS - 1))
        nc.tensor.matmul(dacc[:], pib_c[c][:], wsq_c[c][:],
                         start=(c == 0), stop=(c == NC_CHUNKS - 1))

    def emit_post():
        # attn1 = 1/(1 + dots/(S+1e-8)) on [8,64] diag blocks only
        invs = const.tile([H, 1], F32)
        nc.vector.tensor_scalar_add(invs[:], sacc[:, 0:1], 1e-8)
        nc.vector.reciprocal(invs[:], invs[:])
        t8 = const.tile([H, HD], F32)
        nc.scalar.activation(t8[:], dacc[:], Act.Copy)
        nc.vector.tensor_mul(t8[:], t8[:], maskbd[:])
        d8 = const.tile([H, E], F32)
        nc.vector.tensor_reduce(
            d8[:], t8[:].rearrange("p (hb d) -> p d hb", d=E),
            axis=mybir.AxisListType.X, op=Alu.add)
        nc.vector.tensor_scalar(d8[:], d8[:], invs[:, 0:1], 1.0,
                                op0=Alu.mult, op1=Alu.add)
        nc.vector.reciprocal(d8[:], d8[:])
        # broadcast + negate via negative selector matmuls
        nb_ps = accps.tile([P, HD], F32, tag="bcast")
        for h in range(H):
            nc.tensor.matmul(nb_ps[:, h * E:(h + 1) * E],
                             negsel[:, h * P:(h + 1) * P], d8[:],
                             start=True, stop=True)
        nc.scalar.activation(negattn1b[:], nb_ps[:], Act.Copy)

    def emit_out_chunk(c):
        # out = -(w*Pi)*attn1, fused per head; split DVE/GpSimd for balance
        oc = outp.tile([P, HD], F32, tag="oc")
        for h in range(H):
            nc.vector.scalar_tensor_tensor(
                oc[:, h * E:(h + 1) * E], w_c[c][:, h * E:(h + 1) * E],
                pi_c[c][:, h:h + 1], negattn1b[:, h * E:(h + 1) * E],
                op0=Alu.mult, op1=Alu.mult)
        nc.scalar.dma_start(out_d[c * P:(c + 1) * P, :, :],
                          oc[:].rearrange("p (h d) -> p h d", d=E))

    # ---------------- main G loop ----------------
    # m outer / h inner so head pairs (h, h+1) share one staging tile and one
    # 1 MiB DMA: attn[mP:(m+1)P, h:h+2, :].
    k = 0
    for m in range(NC_CHUNKS):
        for h in range(H):
            # interleaved non-G work, spread through the G pipeline
            if k < 16 and k % 2 == 0:
                emit_pass2_chunk(k // 2)
            elif 16 <= k < 32 and k % 2 == 0:
                emit_sdacc((k - 16) // 2)
            elif k == 33:
                emit_post()
            elif 35 <= k < 35 + 3 * NC_CHUNKS and (k - 35) % 3 == 0:
                emit_out_chunk((k - 35) // 3)

            if h % 2 == 0:
                g2 = gsb.tile([P, 4 * HD], F32, tag="g")
            off = (h % 2) * 2 * HD
            # two PSUM banks per G tile; ACT and DVE evacuate them in parallel
            psa = gps.tile([P, HD], F32, tag="gps", name="psa")
            psb = gps.tile([P, HD], F32, tag="gps", name="psb")
            lhsT = uTv[:, h, m * P:(m + 1) * P]
            nc.tensor.matmul(psa[:], lhsT, uTv[:, h, 0:HD],
                             start=True, stop=True)
            nc.tensor.matmul(psb[:], lhsT, uTv[:, h, HD:2 * HD],
                             start=True, stop=True)
            nc.scalar.activation(g2[:, off:off + HD], psa[:], Act.Identity,
                                 bias=halfcol[:, 0:1], scale=0.5)
            nc.vector.tensor_scalar(g2[:, off + HD:off + 2 * HD], psb[:],
                                    0.5, 0.5, op0=Alu.mult, op1=Alu.add)
            if h % 2 == 1:
                nc.sync.dma_start(
                    attn_d[m * P:(m + 1) * P, h - 1:h + 1, :],
                    g2[:].rearrange("p (t q) -> p t q", q=L))
            k += 1


_CACHE = {}


def _get_nc():
    if "nc" not in _CACHE:
        _CACHE["nc"] = build_kernel()
    return _CACHE["nc"]


def _ensure_trace_support():
    """Register the ctypes NTFF hook + stub out the artifact upload.

    The agent image's antenv lacks axon_hooks, so trn_boot's registration
    degrades silently; recreate the module and register the hook here.
    """
    import types
    import concourse.bass_utils as bu

    bu.upload_artifacts = lambda tmpdir: f"local://{tmpdir}"
    try:
        import antenv.axon_hooks  # noqa: F401
        return
    except ImportError:
        pass
    import antenv
    mod = types.ModuleType("antenv.axon_hooks")
    _h = {}
    mod.set_axon_ntff_profile_hook = lambda hook: _h.__setitem__("hook", hook)
    mod.get_axon_ntff_profile_hook = lambda: _h.get("hook")
    sys.modules["antenv.axon_hooks"] = mod
    antenv.axon_hooks = mod
    from trn_agent_boot.trn_boot import _ntff_profile_via_ctypes
    hook = _ntff_profile_via_ctypes("/opt/axon/libaxon_pjrt.so")
    if hook is not None:
        mod.set_axon_ntff_profile_hook(hook)


def _run(inputs, trace=False):
    from concourse.bass_utils import run_bass_kernel_spmd

    if trace:
        try:
            _ensure_trace_support()
        except Exception as e:  # tracing is best-effort
            print(f"trace support setup failed: {e}")

    queries = np.ascontiguousarray(np.asarray(inputs["queries"], dtype=np.float32))
    temp = np.ascontiguousarray(np.asarray(inputs["temp"], dtype=np.float32))
    assert queries.shape == (B, L, H, E), queries.shape
    assert temp.shape == (H, 1), temp.shape

    nc = _get_nc()
    in_maps = [{"queries": queries[b], "temp": temp} for b in range(N_CORES)]
    res = run_bass_kernel_spmd(nc, in_maps, list(range(N_CORES)), trace=trace)
    out = np.stack([np.asarray(res.results[b]["out"]) for b in range(N_CORES)])
    attn = np.stack([np.asarray(res.results[b]["attn"]) for b in range(N_CORES)])
    return (out, attn), res


def kernel(**inputs):
    (out, attn), _ = _run(inputs, trace=False)
    return out, attn


if __name__ == "__main__":
    nc = build_kernel()
    print("built ok")


# revision 18
# speedup vs baseline: 1.0324x; 1.0324x over previous
"""Trainium2 Bass kernel for nn_AttentionTSSA.

Contract: kernel(**inputs) takes FULL inputs (queries [8,1024,8,64] f32,
temp [8,1] f32) and returns the FULL outputs (out [8,1024,8,64],
attn_reshaped [8,1024,8,1024]) matching reference.reference().

Sharding: batch B=8 across the 8 NeuronCores (pure data parallel, no
collectives). Each core computes one batch element.

Per-core algorithm (n = token index 0..1023, h = head, d = feature):
  w[n, h, d]      = queries[b][n, h, d]               (natural layout)
  wsq             = w*w
  colsumsq[h,d]   = sum_n wsq          (PE ones-matvec -> PSUM [1,512])
  invcol_t[h,d]   = temp[h] / max(colsumsq, 1e-24)
  energy[n,h]     = sum_d wsq[n,h,d] * invcol_t[h,d]  (DVE mul + windowed reduce)
  Pi[n,h]         = softmax_h(energy)                 (DVE/ACT, free-dim softmax)
  S[h]            = sum_n Pi            (PE matvec)
  dots[h,d]       = (sum_n Pi[n,h] wsq[n,h,d]) / (S[h]+1e-8)   (PE matmul, diag blocks)
  attn1[h,d]      = 1/(1+dots)
  out[n,h,d]      = -(w*Pi)*attn1      (DVE scalar_tensor_tensor, fused)
  u[n,h,d]        = w / max(sqrt(sum_d wsq), 1e-12)   (bf16)
  G[h]            = (u[h] @ u[h]^T + 1)*0.5           (PE bf16 matmuls; affine
                                                       folded into PSUM->SBUF copy)
  attn_b[n,h,m]   = G[h][n,m]
"""
import sys
import os
from contextlib import ExitStack

sys.path.insert(0, "/opt/trn_rl_repo")

import numpy as np

import concourse.bass as bass
import concourse.bacc as bacc
import concourse.tile as tile
from concourse import mybir
from concourse.masks import make_identity

F32 = mybir.dt.float32
BF16 = mybir.dt.bfloat16

B, L, H, E = 8, 1024, 8, 64
P = 128                 # SBUF partitions per token chunk
NC_CHUNKS = L // P      # 8 chunks of 128 tokens
HD = H * E              # 512
N_CORES = 8

Alu = mybir.AluOpType
Act = mybir.ActivationFunctionType


def build_kernel():
    nc = bacc.Bacc()
    q_d = nc.declare_dram_parameter("queries", [L, H, E], F32, isOutput=False)
    t_d = nc.declare_dram_parameter("temp", [H, 1], F32, isOutput=False)
    out_d = nc.declare_dram_parameter("out", [L, H, E], F32, isOutput=True)
    attn_d = nc.declare_dram_parameter("attn", [L, H, L], F32, isOutput=True)

    with tile.TileContext(nc) as tc, ExitStack() as ctx:
        _body(ctx, tc, q_d, t_d, out_d, attn_d)
    nc.finalize()
    return nc


def _body(ctx, tc, q_d, t_d, out_d, attn_d):
    nc = tc.nc

    const = ctx.enter_context(tc.tile_pool(name="const", bufs=1))
    persist = ctx.enter_context(tc.tile_pool(name="persist", bufs=1))
    work = ctx.enter_context(tc.tile_pool(name="work", bufs=4))
    small = ctx.enter_context(tc.tile_pool(name="small", bufs=6))
    gsb = ctx.enter_context(tc.tile_pool(name="gsb", bufs=6))
    outp = ctx.enter_context(tc.tile_pool(name="outp", bufs=3))
    accps = ctx.enter_context(tc.tile_pool(name="accps", bufs=1, space="PSUM"))

    # ---------------- constants ----------------
    ident = const.tile([P, P], BF16)
    make_identity(nc, ident[:])
    ones128b = const.tile([P, 1], BF16)
    nc.vector.memset(ones128b[:], 1.0)
    ones128_8b = const.tile([P, H], BF16)
    nc.vector.memset(ones128_8b[:], 1.0)
    halfcol = const.tile([P, 1], F32)
    nc.vector.memset(halfcol[:], 0.5)
    # block-diag 0/1 mask on [8, 512] (head block h lives on partition h)
    maskbd = const.tile([H, HD], F32)
    nc.vector.memset(maskbd[:], 1.0)
    nc.gpsimd.affine_select(
        out=maskbd[:], in_=maskbd[:], compare_op=Alu.is_equal, fill=0.0,
        base=0, pattern=[[1, H], [0, E]], channel_multiplier=-1)
    # selector banks: sel[:, h*128:(h+1)*128] = 1 on partition h, else 0.
    # matmul(sel_h, rhs[8,64]) broadcasts rhs row h to all 128 partitions.
    sel = const.tile([H, H * P], F32)
    nc.vector.memset(sel[:], 1.0)
    nc.gpsimd.affine_select(
        out=sel[:], in_=sel[:], compare_op=Alu.is_equal, fill=0.0,
        base=0, pattern=[[1, H], [0, P]], channel_multiplier=-1)
    negsel = const.tile([H, H * P], F32)
    nc.vector.memset(negsel[:], -1.0)
    nc.gpsimd.affine_select(
        out=negsel[:], in_=negsel[:], compare_op=Alu.is_equal, fill=0.0,
        base=0, pattern=[[1, H], [0, P]], channel_multiplier=-1)
    temp_sb = const.tile([H, 1], F32)
    nc.sync.dma_start(temp_sb[:], t_d[:, :])

    # persistent tensors (per-chunk tiles to keep dep tracking fine-grained)
    w_c = [persist.tile([P, HD], F32, tag=f"w{c}", name=f"w{c}") for c in range(NC_CHUNKS)]
    wsq_c = [persist.tile([P, HD], BF16, tag=f"wsq{c}", name=f"wsq{c}") for c in range(NC_CHUNKS)]
    pi_c = [persist.tile([P, H], F32, tag=f"pi{c}", name=f"pi{c}") for c in range(NC_CHUNKS)]
    pib_c = [persist.tile([P, H], BF16, tag=f"pib{c}", name=f"pib{c}") for c in range(NC_CHUNKS)]
    uT_all = persist.tile([E, H * L], BF16, tag="uT")   # [64, 8192]
    uTv = uT_all[:].rearrange("p (h m) -> p h m", m=L)
    invcolb = persist.tile([P, HD], F32, tag="invcolb")
    negattn1b = persist.tile([P, HD], F32, tag="negattn1b")

    # ---------------- pass 1: load, norms, transposed bf16 u ----------------
    # colsumsq accumulated with M=8 so the result lands on 8 partitions
    # (identical rows); the per-head diag blocks are then extracted to [8,64].
    colacc8 = accps.tile([H, HD], F32, tag="acc")
    with tc.tile_pool(name="trps", bufs=3, space="PSUM") as trps:
        for c in range(NC_CHUNKS):
            nc.sync.dma_start(
                w_c[c][:].rearrange("p (h d) -> p h d", d=E),
                q_d[c * P:(c + 1) * P, :, :],
            )
            nc.scalar.square(wsq_c[c][:], w_c[c][:])
            nc.tensor.matmul(colacc8[:], ones128_8b[:], wsq_c[c][:],
                             start=(c == 0), stop=(c == NC_CHUNKS - 1))
            # row sumsq over d per head -> [128, 8]
            rs = small.tile([P, H], F32, tag="rs")
            nc.vector.tensor_reduce(
                rs[:], wsq_c[c][:].rearrange("p (h d) -> p h d", d=E),
                axis=mybir.AxisListType.X, op=Alu.add)
            nc.scalar.sqrt(rs[:], rs[:])
            nc.vector.tensor_scalar_max(rs[:], rs[:], 1e-12)
            inr = small.tile([P, H], F32, tag="inr")
            nc.vector.reciprocal(inr[:], rs[:])
            # u' = w * invrow (bf16), per head window; split DVE/GpSimd
            ub = work.tile([P, HD], BF16, tag="ub")
            for h in range(H):
                nc.vector.tensor_scalar_mul(
                    ub[:, h * E:(h + 1) * E], w_c[c][:, h * E:(h + 1) * E],
                    inr[:, h:h + 1])
            # transpose each [128, 64] head slice -> PSUM [64, 128]
            trt = trps.tile([E, H * P], BF16, tag="tr")   # [64, 1024]
            for h in range(H):
                nc.tensor.transpose(trt[:, h * P:(h + 1) * P],
                                    ub[:, h * E:(h + 1) * E], ident[:])
            # one strided copy: PSUM [64, 8, 128] -> uT_all[:, h, c*128:+128]
            nc.scalar.activation(
                uTv[:, :, c * P:(c + 1) * P],
                trt[:].rearrange("p (h m) -> p h m", m=P),
                Act.Copy)

    # ---------------- invcol on [8,64] + broadcast ----------------
    gps = ctx.enter_context(tc.tile_pool(name="gps", bufs=5, space="PSUM"))

    # extract diag blocks: mask then strided reduce over the head-block axis
    s8 = const.tile([H, HD], F32)
    nc.scalar.activation(s8[:], colacc8[:], Act.Copy)
    nc.vector.tensor_mul(s8[:], s8[:], maskbd[:])
    c8 = const.tile([H, E], F32)
    nc.vector.tensor_reduce(
        c8[:], s8[:].rearrange("p (hb d) -> p d hb", d=E),
        axis=mybir.AxisListType.X, op=Alu.add)
    nc.vector.tensor_scalar_max(c8[:], c8[:], 1e-24)
    nc.vector.reciprocal(c8[:], c8[:])
    nc.vector.tensor_scalar_mul(c8[:], c8[:], temp_sb[:, 0:1])   # fold temp
    # broadcast row h to all partitions of column block h (8 selector matmuls)
    bc_ps = accps.tile([P, HD], F32, tag="bcast")
    for h in range(H):
        nc.tensor.matmul(bc_ps[:, h * E:(h + 1) * E],
                         sel[:, h * P:(h + 1) * P], c8[:],
                         start=True, stop=True)
    nc.scalar.activation(invcolb[:], bc_ps[:], Act.Copy)

    # ---------------- interleaved work emitted inside the G loop ----------
    sacc = accps.tile([H, HD], F32, tag="acc")
    dacc = accps.tile([H, HD], F32, tag="dacc")

    def emit_pass2_chunk(c):
        # energy + softmax over heads for one token chunk
        et = work.tile([P, HD], F32, tag="et")
        nc.gpsimd.tensor_mul(et[:], wsq_c[c][:], invcolb[:])
        en = small.tile([P, H], F32, tag="en")
        nc.vector.tensor_reduce(
            en[:], et[:].rearrange("p (h d) -> p h d", d=E),
            axis=mybir.AxisListType.X, op=Alu.add)
        nmx = small.tile([P, 1], F32, tag="nmx")
        nc.vector.tensor_reduce(nmx[:], en[:], axis=mybir.AxisListType.X,
                                op=Alu.max, negate=True)
        rsum = small.tile([P, 1], F32, tag="rsum")
        nc.scalar.activation(pi_c[c][:], en[:], Act.Exp,
                             bias=nmx[:, 0:1], scale=1.0, accum_out=rsum[:])
        rinv = small.tile([P, 1], F32, tag="rinv")
        nc.vector.reciprocal(rinv[:], rsum[:])
        nc.vector.tensor_scalar_mul(pi_c[c][:], pi_c[c][:], rinv[:, 0:1])
        nc.vector.tensor_copy(pib_c[c][:], pi_c[c][:])

    def emit_sdacc(c):
        nc.tensor.matmul(sacc[:, 0:1], pib_c[c][:], ones128b[:, 0:1],
                         start=(c == 0), stop=(c == NC_CHUNKS - 1))
        nc.tensor.matmul(dacc[:], pib_c[c][:], wsq_c[c][:],
                         start=(c == 0), stop=(c == NC_CHUNKS - 1))

    def emit_post():
        # attn1 = 1/(1 + dots/(S+1e-8)) on [8,64] diag blocks only
        invs = const.tile([H, 1], F32)
        nc.vector.tensor_scalar_add(invs[:], sacc[:, 0:1], 1e-8)
        nc.vector.reciprocal(invs[:], invs[:])
        t8 = const.tile([H, HD], F32)
        nc.scalar.activation(t8[:], dacc[:], Act.Copy)
        nc.vector.tensor_mul(t8[:], t8[:], maskbd[:])
        d8 = const.tile([H, E], F32)
        nc.vector.tensor_reduce(
            d8[:], t8[:].rearrange("p (hb d) -> p d hb", d=E),
            axis=mybir.AxisListType.X, op=Alu.add)
        nc.vector.tensor_scalar(d8[:], d8[:], invs[:, 0:1], 1.0,
                                op0=Alu.mult, op1=Alu.add)
        nc.vector.reciprocal(d8[:], d8[:])
        # broadcast + negate via negative selector matmuls
        nb_ps = accps.tile([P, HD], F32, tag="bcast")
        for h in range(H):
            nc.tensor.matmul(nb_ps[:, h * E:(h + 1) * E],
                             negsel[:, h * P:(h + 1) * P], d8[:],
                             start=True, stop=True)
        nc.scalar.activation(negattn1b[:], nb_ps[:], Act.Copy)

    def emit_out_chunk(c):
        # out = -(w*Pi)*attn1: wPi on ACT (Identity with per-partition scale),
        # final multiply on GpSimd; keeps DVE free for G evacuation.
        wp = outp.tile([P, HD], F32, tag="wp")
        for h in range(H):
            nc.scalar.activation(wp[:, h * E:(h + 1) * E],
                                 w_c[c][:, h * E:(h + 1) * E],
                                 Act.Identity, scale=pi_c[c][:, h:h + 1])
        oc = outp.tile([P, HD], F32, tag="oc")
        nc.gpsimd.tensor_mul(oc[:], wp[:], negattn1b[:])
        nc.scalar.dma_start(out_d[c * P:(c + 1) * P, :, :],
                          oc[:].rearrange("p (h d) -> p h d", d=E))

    # ---------------- main G loop ----------------
    # m outer / h inner so head pairs (h, h+1) share one staging tile and one
    # 1 MiB DMA: attn[mP:(m+1)P, h:h+2, :].
    k = 0
    for m in range(NC_CHUNKS):
        for h in range(H):
            # interleaved non-G work, spread through the G pipeline
            if k < 16 and k % 2 == 0:
                emit_pass2_chunk(k // 2)
            elif 16 <= k < 32 and k % 2 == 0:
                emit_sdacc((k - 16) // 2)
            elif k == 33:
                emit_post()
            elif 34 <= k < 34 + 4 * NC_CHUNKS and (k - 34) % 4 == 0:
                emit_out_chunk((k - 34) // 4)

            if h % 2 == 0:
                g2 = gsb.tile([P, 4 * HD], F32, tag="g")
            off = (h % 2) * 2 * HD
            # two PSUM banks per G tile; ACT and DVE evacuate them in parallel
            psa = gps.tile([P, HD], F32, tag="gps", name="psa")
            psb = gps.tile([P, HD], F32, tag="gps", name="psb")
            lhsT = uTv[:, h, m * P:(m + 1) * P]
            nc.tensor.matmul(psa[:], lhsT, uTv[:, h, 0:HD],
                             start=True, stop=True)
            nc.tensor.matmul(psb[:], lhsT, uTv[:, h, HD:2 * HD],
                             start=True, stop=True)
            nc.scalar.activation(g2[:, off:off + HD], psa[:], Act.Identity,
                                 bias=halfcol[:, 0:1], scale=0.5)
            nc.vector.tensor_scalar(g2[:, off + HD:off + 2 * HD], psb[:],
                                    0.5, 0.5, op0=Alu.mult, op1=Alu.add)
            if h % 2 == 1:
                nc.sync.dma_start(
                    attn_d[m * P:(m + 1) * P, h - 1:h + 1, :],
                    g2[:].rearrange("p (t q) -> p t q", q=L))
            k += 1


_CACHE = {}


def _get_nc():
    if "nc" not in _CACHE:
        _CACHE["nc"] = build_kernel()
    return _CACHE["nc"]


def _ensure_trace_support():
    """Register the ctypes NTFF hook + stub out the artifact upload.

    The agent image's antenv lacks axon_hooks, so trn_boot's registration
    degrades silently; recreate the module and register the hook here.
    """
    import types
    import concourse.bass_utils as bu

    bu.upload_artifacts = lambda tmpdir: f"local://{tmpdir}"
    try:
        import antenv.axon_hooks  # noqa: F401
        return
    except ImportError:
        pass
    import antenv
    mod = types.ModuleType("antenv.axon_hooks")
    _h = {}
    mod.set_axon_ntff_profile_hook = lambda hook: _h.__setitem__("hook", hook)
    mod.get_axon_ntff_profile_hook = lambda: _h.get("hook")
    sys.modules["antenv.axon_hooks"] = mod
    antenv.axon_hooks = mod
    from trn_agent_boot.trn_boot import _ntff_profile_via_ctypes
    hook = _ntff_profile_via_ctypes("/opt/axon/libaxon_pjrt.so")
    if hook is not None:
        mod.set_axon_ntff_profile_hook(hook)


def _run(inputs, trace=False):
    from concourse.bass_utils import run_bass_kernel_spmd

    if trace:
        try:
            _ensure_trace_support()
        except Exception as e:  # tracing is best-effort
            print(f"trace support setup failed: {e}")

    queries = np.ascontiguousarray(np.asarray(inputs["queries"], dtype=np.float32))
    temp = np.ascontiguousarray(np.asarray(inputs["temp"], dtype=np.float32))
    assert queries.shape == (B, L, H, E), queries.shape
    assert temp.shape == (H, 1), temp.shape

    nc = _get_nc()
    in_maps = [{"queries": queries[b], "temp": temp} for b in range(N_CORES)]
    res = run_bass_kernel_spmd(nc, in_maps, list(range(N_CORES)), trace=trace)
    out = np.stack([np.asarray(res.results[b]["out"]) for b in range(N_CORES)])
    attn = np.stack([np.asarray(res.results[b]["attn"]) for b in range(N_CORES)])
    return (out, attn), res


def kernel(**inputs):
    (out, attn), _ = _run(inputs, trace=False)
    return out, attn


if __name__ == "__main__":
    nc = build_kernel()
    print("built ok")


# revision 19
# speedup vs baseline: 1.0777x; 1.0439x over previous
"""Trainium2 Bass kernel for nn_AttentionTSSA.

Contract: kernel(**inputs) takes FULL inputs (queries [8,1024,8,64] f32,
temp [8,1] f32) and returns the FULL outputs (out [8,1024,8,64],
attn_reshaped [8,1024,8,1024]) matching reference.reference().

Sharding: batch B=8 across the 8 NeuronCores (pure data parallel, no
collectives). Each core computes one batch element.

Per-core algorithm (n = token index 0..1023, h = head, d = feature):
  w[n, h, d]      = queries[b][n, h, d]               (natural layout)
  wsq             = w*w
  colsumsq[h,d]   = sum_n wsq          (PE ones-matvec -> PSUM [1,512])
  invcol_t[h,d]   = temp[h] / max(colsumsq, 1e-24)
  energy[n,h]     = sum_d wsq[n,h,d] * invcol_t[h,d]  (DVE mul + windowed reduce)
  Pi[n,h]         = softmax_h(energy)                 (DVE/ACT, free-dim softmax)
  S[h]            = sum_n Pi            (PE matvec)
  dots[h,d]       = (sum_n Pi[n,h] wsq[n,h,d]) / (S[h]+1e-8)   (PE matmul, diag blocks)
  attn1[h,d]      = 1/(1+dots)
  out[n,h,d]      = -(w*Pi)*attn1      (DVE scalar_tensor_tensor, fused)
  u[n,h,d]        = w / max(sqrt(sum_d wsq), 1e-12)   (bf16)
  G[h]            = (u[h] @ u[h]^T + 1)*0.5           (PE bf16 matmuls; affine
                                                       folded into PSUM->SBUF copy)
  attn_b[n,h,m]   = G[h][n,m]
"""
import sys
import os
from contextlib import ExitStack

sys.path.insert(0, "/opt/trn_rl_repo")

import numpy as np

import concourse.bass as bass
import concourse.bacc as bacc
import concourse.tile as tile
from concourse import mybir
from concourse.masks import make_identity

F32 = mybir.dt.float32
BF16 = mybir.dt.bfloat16

B, L, H, E = 8, 1024, 8, 64
P = 128                 # SBUF partitions per token chunk
NC_CHUNKS = L // P      # 8 chunks of 128 tokens
HD = H * E              # 512
N_CORES = 8

Alu = mybir.AluOpType
Act = mybir.ActivationFunctionType


def build_kernel():
    nc = bacc.Bacc()
    q_d = nc.declare_dram_parameter("queries", [L, H, E], F32, isOutput=False)
    t_d = nc.declare_dram_parameter("temp", [H, 1], F32, isOutput=False)
    out_d = nc.declare_dram_parameter("out", [L, H, E], F32, isOutput=True)
    attn_d = nc.declare_dram_parameter("attn", [L, H, L], F32, isOutput=True)

    with tile.TileContext(nc) as tc, ExitStack() as ctx:
        _body(ctx, tc, q_d, t_d, out_d, attn_d)
    nc.finalize()
    return nc


def _body(ctx, tc, q_d, t_d, out_d, attn_d):
    nc = tc.nc

    const = ctx.enter_context(tc.tile_pool(name="const", bufs=1))
    persist = ctx.enter_context(tc.tile_pool(name="persist", bufs=1))
    work = ctx.enter_context(tc.tile_pool(name="work", bufs=4))
    small = ctx.enter_context(tc.tile_pool(name="small", bufs=6))
    gsb = ctx.enter_context(tc.tile_pool(name="gsb", bufs=4))
    outp = ctx.enter_context(tc.tile_pool(name="outp", bufs=3))
    accps = ctx.enter_context(tc.tile_pool(name="accps", bufs=1, space="PSUM"))

    # ---------------- constants ----------------
    ident = const.tile([P, P], BF16)
    make_identity(nc, ident[:])
    ones128b = const.tile([P, 1], BF16)
    nc.vector.memset(ones128b[:], 1.0)
    ones128_8b = const.tile([P, H], BF16)
    nc.vector.memset(ones128_8b[:], 1.0)
    halfcol = const.tile([P, 1], F32)
    nc.vector.memset(halfcol[:], 0.5)
    # block-diag 0/1 mask on [8, 512] (head block h lives on partition h)
    maskbd = const.tile([H, HD], F32)
    nc.vector.memset(maskbd[:], 1.0)
    nc.gpsimd.affine_select(
        out=maskbd[:], in_=maskbd[:], compare_op=Alu.is_equal, fill=0.0,
        base=0, pattern=[[1, H], [0, E]], channel_multiplier=-1)
    # selector banks: sel[:, h*128:(h+1)*128] = 1 on partition h, else 0.
    # matmul(sel_h, rhs[8,64]) broadcasts rhs row h to all 128 partitions.
    sel = const.tile([H, H * P], F32)
    nc.vector.memset(sel[:], 1.0)
    nc.gpsimd.affine_select(
        out=sel[:], in_=sel[:], compare_op=Alu.is_equal, fill=0.0,
        base=0, pattern=[[1, H], [0, P]], channel_multiplier=-1)
    negsel = const.tile([H, H * P], F32)
    nc.vector.memset(negsel[:], -1.0)
    nc.gpsimd.affine_select(
        out=negsel[:], in_=negsel[:], compare_op=Alu.is_equal, fill=0.0,
        base=0, pattern=[[1, H], [0, P]], channel_multiplier=-1)
    temp_sb = const.tile([H, 1], F32)
    nc.sync.dma_start(temp_sb[:], t_d[:, :])

    # persistent tensors (per-chunk tiles to keep dep tracking fine-grained)
    w_c = [persist.tile([P, HD], F32, tag=f"w{c}", name=f"w{c}") for c in range(NC_CHUNKS)]
    wsq_c = [persist.tile([P, HD], BF16, tag=f"wsq{c}", name=f"wsq{c}") for c in range(NC_CHUNKS)]
    pi_c = [persist.tile([P, H], F32, tag=f"pi{c}", name=f"pi{c}") for c in range(NC_CHUNKS)]
    pib_c = [persist.tile([P, H], BF16, tag=f"pib{c}", name=f"pib{c}") for c in range(NC_CHUNKS)]
    uT_all = persist.tile([E, H * L], BF16, tag="uT")   # [64, 8192]
    uTv = uT_all[:].rearrange("p (h m) -> p h m", m=L)
    invcolb = persist.tile([P, HD], F32, tag="invcolb")
    negattn1b = persist.tile([P, HD], F32, tag="negattn1b")

    # ---------------- pass 1: load, norms, transposed bf16 u ----------------
    # colsumsq accumulated with M=8 so the result lands on 8 partitions
    # (identical rows); the per-head diag blocks are then extracted to [8,64].
    colacc8 = accps.tile([H, HD], F32, tag="acc")
    with tc.tile_pool(name="trps", bufs=3, space="PSUM") as trps:
        for c in range(NC_CHUNKS):
            ring = nc.sync if c % 2 == 0 else nc.scalar
            ring.dma_start(
                w_c[c][:].rearrange("p (h d) -> p h d", d=E),
                q_d[c * P:(c + 1) * P, :, :],
            )
            nc.scalar.square(wsq_c[c][:], w_c[c][:])
            nc.tensor.matmul(colacc8[:], ones128_8b[:], wsq_c[c][:],
                             start=(c == 0), stop=(c == NC_CHUNKS - 1))
            # row sumsq over d per head -> [128, 8]
            rs = small.tile([P, H], F32, tag="rs")
            nc.vector.tensor_reduce(
                rs[:], wsq_c[c][:].rearrange("p (h d) -> p h d", d=E),
                axis=mybir.AxisListType.X, op=Alu.add)
            nc.scalar.sqrt(rs[:], rs[:])
            nc.vector.tensor_scalar_max(rs[:], rs[:], 1e-12)
            inr = small.tile([P, H], F32, tag="inr")
            nc.vector.reciprocal(inr[:], rs[:])
            # u' = w * invrow (bf16), per head window; split DVE/GpSimd
            ub = work.tile([P, HD], BF16, tag="ub")
            for h in range(H):
                nc.vector.tensor_scalar_mul(
                    ub[:, h * E:(h + 1) * E], w_c[c][:, h * E:(h + 1) * E],
                    inr[:, h:h + 1])
            # transpose each [128, 64] head slice -> PSUM [64, 128]
            trt = trps.tile([E, H * P], BF16, tag="tr")   # [64, 1024]
            for h in range(H):
                nc.tensor.transpose(trt[:, h * P:(h + 1) * P],
                                    ub[:, h * E:(h + 1) * E], ident[:])
            # one strided copy: PSUM [64, 8, 128] -> uT_all[:, h, c*128:+128]
            nc.scalar.activation(
                uTv[:, :, c * P:(c + 1) * P],
                trt[:].rearrange("p (h m) -> p h m", m=P),
                Act.Copy)

    # ---------------- invcol on [8,64] + broadcast ----------------
    gps = ctx.enter_context(tc.tile_pool(name="gps", bufs=5, space="PSUM"))

    # extract diag blocks: mask then strided reduce over the head-block axis
    s8 = const.tile([H, HD], F32)
    nc.scalar.activation(s8[:], colacc8[:], Act.Copy)
    nc.vector.tensor_mul(s8[:], s8[:], maskbd[:])
    c8 = const.tile([H, E], F32)
    nc.vector.tensor_reduce(
        c8[:], s8[:].rearrange("p (hb d) -> p d hb", d=E),
        axis=mybir.AxisListType.X, op=Alu.add)
    nc.vector.tensor_scalar_max(c8[:], c8[:], 1e-24)
    nc.vector.reciprocal(c8[:], c8[:])
    nc.vector.tensor_scalar_mul(c8[:], c8[:], temp_sb[:, 0:1])   # fold temp
    # broadcast row h to all partitions of column block h (8 selector matmuls)
    bc_ps = accps.tile([P, HD], F32, tag="bcast")
    for h in range(H):
        nc.tensor.matmul(bc_ps[:, h * E:(h + 1) * E],
                         sel[:, h * P:(h + 1) * P], c8[:],
                         start=True, stop=True)
    nc.scalar.activation(invcolb[:], bc_ps[:], Act.Copy)

    # ---------------- interleaved work emitted inside the G loop ----------
    sacc = accps.tile([H, HD], F32, tag="acc")
    dacc = accps.tile([H, HD], F32, tag="dacc")

    def emit_pass2_chunk(c):
        # energy + softmax over heads for one token chunk
        et = work.tile([P, HD], F32, tag="et")
        nc.gpsimd.tensor_mul(et[:], wsq_c[c][:], invcolb[:])
        en = small.tile([P, H], F32, tag="en")
        nc.vector.tensor_reduce(
            en[:], et[:].rearrange("p (h d) -> p h d", d=E),
            axis=mybir.AxisListType.X, op=Alu.add)
        nmx = small.tile([P, 1], F32, tag="nmx")
        nc.vector.tensor_reduce(nmx[:], en[:], axis=mybir.AxisListType.X,
                                op=Alu.max, negate=True)
        rsum = small.tile([P, 1], F32, tag="rsum")
        nc.scalar.activation(pi_c[c][:], en[:], Act.Exp,
                             bias=nmx[:, 0:1], scale=1.0, accum_out=rsum[:])
        rinv = small.tile([P, 1], F32, tag="rinv")
        nc.vector.reciprocal(rinv[:], rsum[:])
        nc.vector.tensor_scalar_mul(pi_c[c][:], pi_c[c][:], rinv[:, 0:1])
        nc.vector.tensor_copy(pib_c[c][:], pi_c[c][:])

    def emit_sdacc(c):
        nc.tensor.matmul(sacc[:, 0:1], pib_c[c][:], ones128b[:, 0:1],
                         start=(c == 0), stop=(c == NC_CHUNKS - 1))
        nc.tensor.matmul(dacc[:], pib_c[c][:], wsq_c[c][:],
                         start=(c == 0), stop=(c == NC_CHUNKS - 1))

    def emit_post():
        # attn1 = 1/(1 + dots/(S+1e-8)) on [8,64] diag blocks only
        invs = const.tile([H, 1], F32)
        nc.vector.tensor_scalar_add(invs[:], sacc[:, 0:1], 1e-8)
        nc.vector.reciprocal(invs[:], invs[:])
        t8 = const.tile([H, HD], F32)
        nc.scalar.activation(t8[:], dacc[:], Act.Copy)
        nc.vector.tensor_mul(t8[:], t8[:], maskbd[:])
        d8 = const.tile([H, E], F32)
        nc.vector.tensor_reduce(
            d8[:], t8[:].rearrange("p (hb d) -> p d hb", d=E),
            axis=mybir.AxisListType.X, op=Alu.add)
        nc.vector.tensor_scalar(d8[:], d8[:], invs[:, 0:1], 1.0,
                                op0=Alu.mult, op1=Alu.add)
        nc.vector.reciprocal(d8[:], d8[:])
        # broadcast + negate via negative selector matmuls
        nb_ps = accps.tile([P, HD], F32, tag="bcast")
        for h in range(H):
            nc.tensor.matmul(nb_ps[:, h * E:(h + 1) * E],
                             negsel[:, h * P:(h + 1) * P], d8[:],
                             start=True, stop=True)
        nc.scalar.activation(negattn1b[:], nb_ps[:], Act.Copy)

    def emit_out_chunk(c):
        # out = -(w*Pi)*attn1: wPi on ACT (Identity with per-partition scale),
        # final multiply on GpSimd; keeps DVE free for G evacuation.
        wp = outp.tile([P, HD], F32, tag="wp")
        for h in range(H):
            nc.scalar.activation(wp[:, h * E:(h + 1) * E],
                                 w_c[c][:, h * E:(h + 1) * E],
                                 Act.Identity, scale=pi_c[c][:, h:h + 1])
        oc = outp.tile([P, HD], F32, tag="oc")
        nc.gpsimd.tensor_mul(oc[:], wp[:], negattn1b[:])
        nc.scalar.dma_start(out_d[c * P:(c + 1) * P, :, :],
                          oc[:].rearrange("p (h d) -> p h d", d=E))

    # ---------------- main G loop ----------------
    # m outer / h inner; four heads share one staging tile and one 2 MiB DMA
    # (fewer per-DMA completion stalls); DMAs alternate between the two HWDGE
    # rings so one ring's completion receipt hides behind the other's queue.
    k = 0
    quad = 0
    for m in range(NC_CHUNKS):
        for h in range(H):
            # interleaved non-G work, spread through the G pipeline
            if k < 16 and k % 2 == 0:
                emit_pass2_chunk(k // 2)
            elif 16 <= k < 32 and k % 2 == 0:
                emit_sdacc((k - 16) // 2)
            elif k == 33:
                emit_post()
            elif 34 <= k < 34 + 4 * NC_CHUNKS and (k - 34) % 4 == 0:
                emit_out_chunk((k - 34) // 4)

            if h % 4 == 0:
                g4 = gsb.tile([P, 8 * HD], F32, tag="g")
            off = (h % 4) * 2 * HD
            # two PSUM banks per G tile; ACT and DVE evacuate them in parallel
            psa = gps.tile([P, HD], F32, tag="gps", name="psa")
            psb = gps.tile([P, HD], F32, tag="gps", name="psb")
            lhsT = uTv[:, h, m * P:(m + 1) * P]
            nc.tensor.matmul(psa[:], lhsT, uTv[:, h, 0:HD],
                             start=True, stop=True)
            nc.tensor.matmul(psb[:], lhsT, uTv[:, h, HD:2 * HD],
                             start=True, stop=True)
            nc.scalar.activation(g4[:, off:off + HD], psa[:], Act.Identity,
                                 bias=halfcol[:, 0:1], scale=0.5)
            nc.vector.tensor_scalar(g4[:, off + HD:off + 2 * HD], psb[:],
                                    0.5, 0.5, op0=Alu.mult, op1=Alu.add)
            if h % 4 == 3:
                ring = nc.sync if quad % 2 == 0 else nc.scalar
                ring.dma_start(
                    attn_d[m * P:(m + 1) * P, h - 3:h + 1, :],
                    g4[:].rearrange("p (t q) -> p t q", q=L))
                quad += 1
            k += 1


_CACHE = {}


def _get_nc():
    if "nc" not in _CACHE:
        _CACHE["nc"] = build_kernel()
    return _CACHE["nc"]


def _ensure_trace_support():
    """Register the ctypes NTFF hook + stub out the artifact upload.

    The agent image's antenv lacks axon_hooks, so trn_boot's registration
    degrades silently; recreate the module and register the hook here.
    """
    import types
    import concourse.bass_utils as bu

    bu.upload_artifacts = lambda tmpdir: f"local://{tmpdir}"
    try:
        import antenv.axon_hooks  # noqa: F401
        return
    except ImportError:
        pass
    import antenv
    mod = types.ModuleType("antenv.axon_hooks")
    _h = {}
    mod.set_axon_ntff_profile_hook = lambda hook: _h.__setitem__("hook", hook)
    mod.get_axon_ntff_profile_hook = lambda: _h.get("hook")
    sys.modules["antenv.axon_hooks"] = mod
    antenv.axon_hooks = mod
    from trn_agent_boot.trn_boot import _ntff_profile_via_ctypes
    hook = _ntff_profile_via_ctypes("/opt/axon/libaxon_pjrt.so")
    if hook is not None:
        mod.set_axon_ntff_profile_hook(hook)


def _run(inputs, trace=False):
    from concourse.bass_utils import run_bass_kernel_spmd

    if trace:
        try:
            _ensure_trace_support()
        except Exception as e:  # tracing is best-effort
            print(f"trace support setup failed: {e}")

    queries = np.ascontiguousarray(np.asarray(inputs["queries"], dtype=np.float32))
    temp = np.ascontiguousarray(np.asarray(inputs["temp"], dtype=np.float32))
    assert queries.shape == (B, L, H, E), queries.shape
    assert temp.shape == (H, 1), temp.shape

    nc = _get_nc()
    in_maps = [{"queries": queries[b], "temp": temp} for b in range(N_CORES)]
    res = run_bass_kernel_spmd(nc, in_maps, list(range(N_CORES)), trace=trace)
    out = np.stack([np.asarray(res.results[b]["out"]) for b in range(N_CORES)])
    attn = np.stack([np.asarray(res.results[b]["attn"]) for b in range(N_CORES)])
    return (out, attn), res


def kernel(**inputs):
    (out, attn), _ = _run(inputs, trace=False)
    return out, attn


if __name__ == "__main__":
    nc = build_kernel()
    print("built ok")


# revision 20
# speedup vs baseline: 1.1881x; 1.1024x over previous
"""Trainium2 Bass kernel for nn_AttentionTSSA.

Contract: kernel(**inputs) takes FULL inputs (queries [8,1024,8,64] f32,
temp [8,1] f32) and returns the FULL outputs (out [8,1024,8,64],
attn_reshaped [8,1024,8,1024]) matching reference.reference().

Sharding: batch B=8 across the 8 NeuronCores (pure data parallel, no
collectives). Each core computes one batch element.

Per-core algorithm (n = token index 0..1023, h = head, d = feature):
  w[n, h, d]      = queries[b][n, h, d]               (natural layout)
  wsq             = w*w
  colsumsq[h,d]   = sum_n wsq          (PE ones-matvec -> PSUM [1,512])
  invcol_t[h,d]   = temp[h] / max(colsumsq, 1e-24)
  energy[n,h]     = sum_d wsq[n,h,d] * invcol_t[h,d]  (DVE mul + windowed reduce)
  Pi[n,h]         = softmax_h(energy)                 (DVE/ACT, free-dim softmax)
  S[h]            = sum_n Pi            (PE matvec)
  dots[h,d]       = (sum_n Pi[n,h] wsq[n,h,d]) / (S[h]+1e-8)   (PE matmul, diag blocks)
  attn1[h,d]      = 1/(1+dots)
  out[n,h,d]      = -(w*Pi)*attn1      (DVE scalar_tensor_tensor, fused)
  u[n,h,d]        = w / max(sqrt(sum_d wsq), 1e-12)   (bf16)
  G[h]            = (u[h] @ u[h]^T + 1)*0.5           (PE bf16 matmuls; affine
                                                       folded into PSUM->SBUF copy)
  attn_b[n,h,m]   = G[h][n,m]
"""
import sys
import os
from contextlib import ExitStack

sys.path.insert(0, "/opt/trn_rl_repo")

import numpy as np

import concourse.bass as bass
import concourse.bacc as bacc
import concourse.tile as tile
from concourse import mybir
from concourse.masks import make_identity

F32 = mybir.dt.float32
BF16 = mybir.dt.bfloat16

B, L, H, E = 8, 1024, 8, 64
P = 128                 # SBUF partitions per token chunk
NC_CHUNKS = L // P      # 8 chunks of 128 tokens
HD = H * E              # 512
N_CORES = 8

Alu = mybir.AluOpType
Act = mybir.ActivationFunctionType


def build_kernel():
    nc = bacc.Bacc()
    q_d = nc.declare_dram_parameter("queries", [L, H, E], F32, isOutput=False)
    t_d = nc.declare_dram_parameter("temp", [H, 1], F32, isOutput=False)
    out_d = nc.declare_dram_parameter("out", [L, H, E], F32, isOutput=True)
    attn_d = nc.declare_dram_parameter("attn", [L, H, L], F32, isOutput=True)

    with tile.TileContext(nc) as tc, ExitStack() as ctx:
        _body(ctx, tc, q_d, t_d, out_d, attn_d)
    nc.finalize()
    return nc


def _body(ctx, tc, q_d, t_d, out_d, attn_d):
    nc = tc.nc

    const = ctx.enter_context(tc.tile_pool(name="const", bufs=1))
    persist = ctx.enter_context(tc.tile_pool(name="persist", bufs=1))
    work = ctx.enter_context(tc.tile_pool(name="work", bufs=4))
    small = ctx.enter_context(tc.tile_pool(name="small", bufs=6))
    gsb = ctx.enter_context(tc.tile_pool(name="gsb", bufs=4))
    outp = ctx.enter_context(tc.tile_pool(name="outp", bufs=3))
    accps = ctx.enter_context(tc.tile_pool(name="accps", bufs=1, space="PSUM"))

    # ---------------- constants ----------------
    ident = const.tile([P, P], BF16)
    make_identity(nc, ident[:])
    ones128b = const.tile([P, 1], BF16)
    nc.vector.memset(ones128b[:], 1.0)
    ones128_8b = const.tile([P, H], BF16)
    nc.vector.memset(ones128_8b[:], 1.0)
    halfcol = const.tile([P, 1], F32)
    nc.vector.memset(halfcol[:], 0.5)
    # block-diag 0/1 mask on [8, 512] (head block h lives on partition h)
    maskbd = const.tile([H, HD], F32)
    nc.vector.memset(maskbd[:], 1.0)
    nc.gpsimd.affine_select(
        out=maskbd[:], in_=maskbd[:], compare_op=Alu.is_equal, fill=0.0,
        base=0, pattern=[[1, H], [0, E]], channel_multiplier=-1)
    # selector banks: sel[:, h*128:(h+1)*128] = 1 on partition h, else 0.
    # matmul(sel_h, rhs[8,64]) broadcasts rhs row h to all 128 partitions.
    sel = const.tile([H, H * P], F32)
    nc.vector.memset(sel[:], 1.0)
    nc.gpsimd.affine_select(
        out=sel[:], in_=sel[:], compare_op=Alu.is_equal, fill=0.0,
        base=0, pattern=[[1, H], [0, P]], channel_multiplier=-1)
    negsel = const.tile([H, H * P], F32)
    nc.vector.memset(negsel[:], -1.0)
    nc.gpsimd.affine_select(
        out=negsel[:], in_=negsel[:], compare_op=Alu.is_equal, fill=0.0,
        base=0, pattern=[[1, H], [0, P]], channel_multiplier=-1)
    temp_sb = const.tile([H, 1], F32)
    nc.sync.dma_start(temp_sb[:], t_d[:, :])

    # persistent tensors (per-chunk tiles to keep dep tracking fine-grained)
    w_c = [persist.tile([P, HD], F32, tag=f"w{c}", name=f"w{c}") for c in range(NC_CHUNKS)]
    wsq_c = [persist.tile([P, HD], BF16, tag=f"wsq{c}", name=f"wsq{c}") for c in range(NC_CHUNKS)]
    pi_c = [persist.tile([P, H], F32, tag=f"pi{c}", name=f"pi{c}") for c in range(NC_CHUNKS)]
    pib_c = [persist.tile([P, H], BF16, tag=f"pib{c}", name=f"pib{c}") for c in range(NC_CHUNKS)]
    uT_all = persist.tile([E, H * L], BF16, tag="uT")   # [64, 8192]
    uTv = uT_all[:].rearrange("p (h m) -> p h m", m=L)
    invcolb = persist.tile([P, HD], F32, tag="invcolb")
    negattn1b = persist.tile([P, HD], F32, tag="negattn1b")

    # ---------------- pass 1: load, norms, transposed bf16 u ----------------
    # colsumsq accumulated with M=8 so the result lands on 8 partitions
    # (identical rows); the per-head diag blocks are then extracted to [8,64].
    colacc8 = accps.tile([H, HD], F32, tag="acc")
    with tc.tile_pool(name="trps", bufs=3, space="PSUM") as trps:
        for c in range(NC_CHUNKS):
            ring = nc.sync if c % 2 == 0 else nc.scalar
            ring.dma_start(
                w_c[c][:].rearrange("p (h d) -> p h d", d=E),
                q_d[c * P:(c + 1) * P, :, :],
            )
            nc.scalar.square(wsq_c[c][:], w_c[c][:])
            nc.tensor.matmul(colacc8[:], ones128_8b[:], wsq_c[c][:],
                             start=(c == 0), stop=(c == NC_CHUNKS - 1))
            # row sumsq over d per head -> [128, 8]
            rs = small.tile([P, H], F32, tag="rs")
            nc.vector.tensor_reduce(
                rs[:], wsq_c[c][:].rearrange("p (h d) -> p h d", d=E),
                axis=mybir.AxisListType.X, op=Alu.add)
            nc.scalar.sqrt(rs[:], rs[:])
            nc.vector.tensor_scalar_max(rs[:], rs[:], 1e-12)
            inr = small.tile([P, H], F32, tag="inr")
            nc.vector.reciprocal(inr[:], rs[:])
            # u' = w * invrow (bf16), per head window; split DVE/GpSimd
            ub = work.tile([P, HD], BF16, tag="ub")
            for h in range(H):
                nc.vector.tensor_scalar_mul(
                    ub[:, h * E:(h + 1) * E], w_c[c][:, h * E:(h + 1) * E],
                    inr[:, h:h + 1])
            # transpose each [128, 64] head slice -> PSUM [64, 128]
            trt = trps.tile([E, H * P], BF16, tag="tr")   # [64, 1024]
            for h in range(H):
                nc.tensor.transpose(trt[:, h * P:(h + 1) * P],
                                    ub[:, h * E:(h + 1) * E], ident[:])
            # one strided copy: PSUM [64, 8, 128] -> uT_all[:, h, c*128:+128]
            nc.scalar.activation(
                uTv[:, :, c * P:(c + 1) * P],
                trt[:].rearrange("p (h m) -> p h m", m=P),
                Act.Copy)

    # ---------------- invcol on [8,64] + broadcast ----------------
    gps = ctx.enter_context(tc.tile_pool(name="gps", bufs=5, space="PSUM"))

    # extract diag blocks: mask then strided reduce over the head-block axis
    s8 = const.tile([H, HD], F32)
    nc.scalar.activation(s8[:], colacc8[:], Act.Copy)
    nc.vector.tensor_mul(s8[:], s8[:], maskbd[:])
    c8 = const.tile([H, E], F32)
    nc.vector.tensor_reduce(
        c8[:], s8[:].rearrange("p (hb d) -> p d hb", d=E),
        axis=mybir.AxisListType.X, op=Alu.add)
    nc.vector.tensor_scalar_max(c8[:], c8[:], 1e-24)
    nc.vector.reciprocal(c8[:], c8[:])
    nc.vector.tensor_scalar_mul(c8[:], c8[:], temp_sb[:, 0:1])   # fold temp
    # broadcast row h to all partitions of column block h (8 selector matmuls)
    bc_ps = accps.tile([P, HD], F32, tag="bcast")
    for h in range(H):
        nc.tensor.matmul(bc_ps[:, h * E:(h + 1) * E],
                         sel[:, h * P:(h + 1) * P], c8[:],
                         start=True, stop=True)
    nc.scalar.activation(invcolb[:], bc_ps[:], Act.Copy)

    # ---------------- interleaved work emitted inside the G loop ----------
    sacc = accps.tile([H, HD], F32, tag="acc")
    dacc = accps.tile([H, HD], F32, tag="dacc")

    def emit_pass2_chunk(c):
        # energy + softmax over heads for one token chunk
        et = work.tile([P, HD], F32, tag="et")
        nc.gpsimd.tensor_mul(et[:], wsq_c[c][:], invcolb[:])
        en = small.tile([P, H], F32, tag="en")
        nc.vector.tensor_reduce(
            en[:], et[:].rearrange("p (h d) -> p h d", d=E),
            axis=mybir.AxisListType.X, op=Alu.add)
        rsum = small.tile([P, 1], F32, tag="rsum")
        nc.scalar.activation(pi_c[c][:], en[:], Act.Exp,
                             scale=1.0, accum_out=rsum[:])
        rinv = small.tile([P, 1], F32, tag="rinv")
        nc.vector.reciprocal(rinv[:], rsum[:])
        nc.vector.tensor_scalar_mul(pi_c[c][:], pi_c[c][:], rinv[:, 0:1])
        nc.vector.tensor_copy(pib_c[c][:], pi_c[c][:])

    def emit_sdacc(c):
        nc.tensor.matmul(sacc[:, 0:1], pib_c[c][:], ones128b[:, 0:1],
                         start=(c == 0), stop=(c == NC_CHUNKS - 1))
        nc.tensor.matmul(dacc[:], pib_c[c][:], wsq_c[c][:],
                         start=(c == 0), stop=(c == NC_CHUNKS - 1))

    def emit_post():
        # attn1 = 1/(1 + dots/(S+1e-8)) on [8,64] diag blocks only
        invs = const.tile([H, 1], F32)
        nc.vector.tensor_scalar_add(invs[:], sacc[:, 0:1], 1e-8)
        nc.vector.reciprocal(invs[:], invs[:])
        t8 = const.tile([H, HD], F32)
        nc.scalar.activation(t8[:], dacc[:], Act.Copy)
        nc.vector.tensor_mul(t8[:], t8[:], maskbd[:])
        d8 = const.tile([H, E], F32)
        nc.vector.tensor_reduce(
            d8[:], t8[:].rearrange("p (hb d) -> p d hb", d=E),
            axis=mybir.AxisListType.X, op=Alu.add)
        nc.vector.tensor_scalar(d8[:], d8[:], invs[:, 0:1], 1.0,
                                op0=Alu.mult, op1=Alu.add)
        nc.vector.reciprocal(d8[:], d8[:])
        # broadcast + negate via negative selector matmuls
        nb_ps = accps.tile([P, HD], F32, tag="bcast")
        for h in range(H):
            nc.tensor.matmul(nb_ps[:, h * E:(h + 1) * E],
                             negsel[:, h * P:(h + 1) * P], d8[:],
                             start=True, stop=True)
        nc.scalar.activation(negattn1b[:], nb_ps[:], Act.Copy)

    def emit_out_chunk(c):
        # out = -(w*Pi)*attn1, fused per head on DVE
        oc = outp.tile([P, HD], F32, tag="oc")
        for h in range(H):
            nc.vector.scalar_tensor_tensor(
                oc[:, h * E:(h + 1) * E], w_c[c][:, h * E:(h + 1) * E],
                pi_c[c][:, h:h + 1], negattn1b[:, h * E:(h + 1) * E],
                op0=Alu.mult, op1=Alu.mult)
        nc.scalar.dma_start(out_d[c * P:(c + 1) * P, :, :],
                          oc[:].rearrange("p (h d) -> p h d", d=E))

    # ---------------- main G loop ----------------
    # m outer / h inner; four heads share one staging tile and one 2 MiB DMA
    # (fewer per-DMA completion stalls); DMAs alternate between the two HWDGE
    # rings so one ring's completion receipt hides behind the other's queue.
    k = 0
    quad = 0
    for m in range(NC_CHUNKS):
        for h in range(H):
            # interleaved non-G work, spread through the G pipeline
            if k < 32 and k % 4 == 0:
                emit_pass2_chunk(k // 4)
            elif k < 32 and k % 4 == 2:
                emit_sdacc((k - 2) // 4)
            elif k == 33:
                emit_post()
            elif 35 <= k < 35 + 3 * NC_CHUNKS and (k - 35) % 3 == 0:
                emit_out_chunk((k - 35) // 3)

            if h % 4 == 0:
                g4 = gsb.tile([P, 8 * HD], F32, tag="g")
            off = (h % 4) * 2 * HD
            # two PSUM banks per G tile; ACT and DVE evacuate them in parallel
            psa = gps.tile([P, HD], F32, tag="gps", name="psa")
            psb = gps.tile([P, HD], F32, tag="gps", name="psb")
            lhsT = uTv[:, h, m * P:(m + 1) * P]
            nc.tensor.matmul(psa[:], lhsT, uTv[:, h, 0:HD],
                             start=True, stop=True)
            nc.tensor.matmul(psb[:], lhsT, uTv[:, h, HD:2 * HD],
                             start=True, stop=True)
            nc.scalar.activation(g4[:, off:off + HD], psa[:], Act.Identity,
                                 bias=halfcol[:, 0:1], scale=0.5)
            nc.vector.tensor_scalar(g4[:, off + HD:off + 2 * HD], psb[:],
                                    0.5, 0.5, op0=Alu.mult, op1=Alu.add)
            if h % 4 == 3:
                ring = nc.sync if quad % 2 == 0 else nc.scalar
                ring.dma_start(
                    attn_d[m * P:(m + 1) * P, h - 3:h + 1, :],
                    g4[:].rearrange("p (t q) -> p t q", q=L))
                quad += 1
            k += 1


_CACHE = {}


def _get_nc():
    if "nc" not in _CACHE:
        _CACHE["nc"] = build_kernel()
    return _CACHE["nc"]


def _ensure_trace_support():
    """Register the ctypes NTFF hook + stub out the artifact upload.

    The agent image's antenv lacks axon_hooks, so trn_boot's registration
    degrades silently; recreate the module and register the hook here.
    """
    import types
    import concourse.bass_utils as bu

    bu.upload_artifacts = lambda tmpdir: f"local://{tmpdir}"
    try:
        import antenv.axon_hooks  # noqa: F401
        return
    except ImportError:
        pass
    import antenv
    mod = types.ModuleType("antenv.axon_hooks")
    _h = {}
    mod.set_axon_ntff_profile_hook = lambda hook: _h.__setitem__("hook", hook)
    mod.get_axon_ntff_profile_hook = lambda: _h.get("hook")
    sys.modules["antenv.axon_hooks"] = mod
    antenv.axon_hooks = mod
    from trn_agent_boot.trn_boot import _ntff_profile_via_ctypes
    hook = _ntff_profile_via_ctypes("/opt/axon/libaxon_pjrt.so")
    if hook is not None:
        mod.set_axon_ntff_profile_hook(hook)


def _run(inputs, trace=False):
    from concourse.bass_utils import run_bass_kernel_spmd

    if trace:
        try:
            _ensure_trace_support()
        except Exception as e:  # tracing is best-effort
            print(f"trace support setup failed: {e}")

    queries = np.ascontiguousarray(np.asarray(inputs["queries"], dtype=np.float32))
    temp = np.ascontiguousarray(np.asarray(inputs["temp"], dtype=np.float32))
    assert queries.shape == (B, L, H, E), queries.shape
    assert temp.shape == (H, 1), temp.shape

    nc = _get_nc()
    in_maps = [{"queries": queries[b], "temp": temp} for b in range(N_CORES)]
    res = run_bass_kernel_spmd(nc, in_maps, list(range(N_CORES)), trace=trace)
    out = np.stack([np.asarray(res.results[b]["out"]) for b in range(N_CORES)])
    attn = np.stack([np.asarray(res.results[b]["attn"]) for b in range(N_CORES)])
    return (out, attn), res


def kernel(**inputs):
    (out, attn), _ = _run(inputs, trace=False)
    return out, attn


if __name__ == "__main__":
    nc = build_kernel()
    print("built ok")
